# revision 17
# baseline (speedup 1.0000x reference)
"""Multi-head attention forward, sharded over 8 NeuronCores.

Sharding: batch (2) x head-group (4 groups of 4 heads) = 8 cores.
Each core computes its batch's 4 heads end-to-end; the out-projection
produces a partial [S, HID] that the host sums across the 4 head
groups per batch (and adds out_b).

Per-core pipeline (all bf16 matmuls, fp32 psum):
  - Q^T/K^T = W^T-slices @ x^T (psum), bias folded into the psum->SBUF
    copy via tensor_scalar (per-partition bias AP); K/V computed on
    mask-compacted keys only (KC ~ 1152 of 2048, padded to 128).
  - scores S^T[k,q] per head = K-slice.T @ Q-slice (64-contraction,
    PE-array tile packing via tile_position).
  - P^T = exp(0.125*S^T + maskbias) on ACT (maskbias is a per-partition
    bias AP; compacted padding gets -50 -> P=0).
  - ctx_aug^T = [V|1]^T.T @ P^T -- the ones column yields softmax
    denominators for free; V stored token-major with 4 head blocks.
  - normalization: reciprocal (DVE) -> partition_broadcast (GpSimd) ->
    tensor_tensor multiply psum x bcast -> ctxT bf16 (DVE).
  - out-projection runs as a tail phase (interleaving it into the
    attention loop measured slower); psum->SBUF copies on ACT (idle in
    the tail), output DMA'd as bf16 partials.
Scheduling: the QKV projection is software-pipelined INTO the attention
loop (K/Q chunk 0 as prologue; V tiles and remaining K/Q chunks drained
between kt slots by need-time), PV lags scores/exp by PV_LAG slots, and
norm runs on due-slots -- all to keep every engine's queue pre-satisfied
(blocking cross-engine waits measured ~2-4us on this axon path).
An fp8 DoubleRow scores path (SCORES_BF16=0) exists but measured slower
on hardware than bf16 scores despite the 0.5 cycles/row model (DR
ldweights are not overlapped), and costs ~1.2e-2 extra error.
"""

import os
import sys

if "/opt/trn_rl_repo" not in sys.path:
    sys.path.insert(0, "/opt/trn_rl_repo")

import numpy as np
import ml_dtypes

import concourse.bass as bass
import concourse.mybir as mybir
from concourse import bacc
from concourse.bass import ts, ds
from concourse.tile import TileContext
from concourse import bass_utils

BF16 = mybir.dt.bfloat16
F32 = mybir.dt.float32
F32R = mybir.dt.float32r
FP8 = mybir.dt.float8e4
EXP = mybir.ActivationFunctionType.Exp
MULT = mybir.AluOpType.mult
ADD = mybir.AluOpType.add
DR = mybir.MatmulPerfMode.DoubleRow
E4 = ml_dtypes.float8_e4m3

N_CORES = 8
S = 2048          # sequence length (one batch per core)
HID = 1024
DH = 256          # head dims per core (4 heads x 64)
D = 64
NEG = -50.0       # additive mask bias (post-scale); exp(-50) ~ 2e-22
KC = 1152         # compacted+padded key length; ~1034 unmasked for seed 0
NKT = KC // 128
SCORES_BF16 = bool(int(os.environ.get("SCORES_BF16", "1")))
QS = 1.0 if SCORES_BF16 else float(os.environ.get("QS", "4"))


def build_program(reps=1):
    nkt = NKT
    nc = bacc.Bacc("TRN2", target_bir_lowering=False, debug=False,
                   num_devices=N_CORES)
    def dram(name, shape, dt):
        return nc.dram_tensor(name, shape, dt, kind="ExternalInput").ap()
    xtb = dram("xtb", [HID, S], BF16)
    xkb = dram("xkb", [HID, KC], BF16)
    wqT = dram("wqT", [HID, 2, 128], BF16)
    wkT = dram("wkT", [HID, 2, 128], BF16)
    wvT = dram("wvT", [HID, DH], BF16)
    qkb = dram("qkb", [128, 4], F32)
    vb = dram("vb", [1, DH], BF16)
    woT = dram("woT", [DH, HID], BF16)
    maskb = dram("maskb", [128, NKT], F32)
    out = nc.dram_tensor("out", [S, HID], BF16, kind="ExternalOutput").ap()

    with TileContext(nc) as tc:
        with tc.tile_pool(name="const", bufs=1) as cp:
            wq_sb = cp.tile([128, 8, 2, 128], BF16, name="wq_sb")
            wk_sb = cp.tile([128, 8, 2, 128], BF16, name="wk_sb")
            nc.sync.dma_start(wk_sb,
                              wkT.rearrange("(c p) s m -> p c s m", p=128))
            nc.sync.dma_start(wq_sb,
                              wqT.rearrange("(c p) s m -> p c s m", p=128))
            wv_sb = cp.tile([128, 8, DH], BF16, name="wv_sb")
            nc.sync.dma_start(wv_sb,
                              wvT.rearrange("(c p) m -> p c m", p=128))
            wo_sb = cp.tile([128, 2, HID], BF16, name="wo_sb")
            nc.sync.dma_start(wo_sb, woT.rearrange("(c p) o -> p c o", p=128))
            qkb_sb = cp.tile([128, 4], F32, name="qkb_sb")
            nc.sync.dma_start(qkb_sb, qkb)
            vb_sb = cp.tile([1, DH], BF16, name="vb_sb")
            nc.sync.dma_start(vb_sb, vb)
            vb_f = cp.tile([1, DH], F32, name="vb_f")
            nc.vector.tensor_copy(vb_f, vb_sb)
            vb_bc = cp.tile([128, DH], F32, name="vb_bc")
            nc.gpsimd.partition_broadcast(vb_bc, vb_f)
            maskb_sb = cp.tile([128, NKT], F32, name="maskb_sb")
            nc.sync.dma_start(maskb_sb, maskb)
            ones_bf = cp.tile([1, 128], BF16, name="ones_bf")
            nc.vector.memset(ones_bf, 1.0)
            ones_tmp = cp.tile([1, 64], F32, name="ones_tmp")
            nc.vector.memset(ones_tmp, 1.0)
            ones_f32 = cp.tile([1, 64], F32R, name="ones_f32")
            with nc.allow_low_precision(reason="f32r ones"):
                nc.vector.tensor_copy(ones_f32, ones_tmp)

            xk_sb = cp.tile([128, 8, KC], BF16, name="xk_sb")
            view = xkb.rearrange("(c p) t -> c p t", p=128)
            for c in range(8):
                nc.sync.dma_start(xk_sb[:, c, :], view[c])
            xt_sb = cp.tile([128, 8, S], BF16, name="xt_sb")
            view = xtb.rearrange("(c p) t -> c p t", p=128)
            for c in range(8):
                nc.sync.dma_start(xt_sb[:, c, :], view[c])

            qk_dt = BF16 if SCORES_BF16 else FP8
            q_il = cp.tile([128, 2, S], qk_dt, name="q_il")
            k_il = cp.tile([128, 2, KC], qk_dt, name="k_il")
            v_sb = cp.tile([128, nkt, 4 * 65], BF16, name="v_sb")
            ctxT = [cp.tile([128, S], BF16, name=f"ctxT{j}") for j in range(2)]
            for h in range(4):
                nc.vector.memset(v_sb[:, :, 65 * h + 64:65 * h + 65], 1.0)

            for _rep in range(reps):
                with tc.tile_pool(name="psS",
                                  bufs=int(os.environ.get("PSS_BUFS", "2")),
                                  space="PSUM") as psS, \
                     tc.tile_pool(name="psX",
                                  bufs=int(os.environ.get("PSX_BUFS", "4")),
                                  space="PSUM") as psX, \
                     tc.tile_pool(name="ptp",
                              bufs=int(os.environ.get("PT_BUFS", "6"))) as ptp, \
                     tc.tile_pool(name="npool", bufs=2) as npool, \
                     tc.tile_pool(name="outp", bufs=3) as outp:

                    def emit_qk(dst, w_sb, x_sb, s, off, w, bias_col):
                        ps = psS.tile([128, 512], F32, name="ps_qk",
                                      tag="s_ps")
                        for c in range(8):
                            nc.tensor.matmul(
                                ps[:, 0:w],
                                lhsT=w_sb[:, c, s, :],
                                rhs=x_sb[:, c, ds(off, w)],
                                start=(c == 0), stop=(c == 7))
                        nc.vector.tensor_scalar(
                            out=dst[:, s, ds(off, w)], in0=ps[:, 0:w],
                            scalar1=QS,
                            scalar2=qkb_sb[:, bias_col:bias_col + 1],
                            op0=MULT, op1=ADD)

                    def emit_v(i):
                        ps = psS.tile([128, DH], F32, name="ps_v",
                                      tag="s_ps")
                        for c in range(8):
                            nc.tensor.matmul(ps,
                                             lhsT=xk_sb[:, c, ts(i, 128)],
                                             rhs=wv_sb[:, c, :],
                                             start=(c == 0), stop=(c == 7))
                        nc.vector.tensor_tensor(
                            out=v_sb[:, i, :].rearrange(
                                "p (h c) -> p h c", c=65)[:, :, 0:64],
                            in0=ps.rearrange("p (h c) -> p h c", c=64),
                            in1=vb_bc.rearrange("p (h c) -> p h c", c=64),
                            op=ADD)

                    def emit_pv(job):
                        jhp, jctxA, jctxB, jpt, jkt = job[:5]
                        for h, jctx, col in ((2 * jhp, jctxA, 0),
                                             (2 * jhp + 1, jctxB, 1)):
                            nc.tensor.matmul(
                                jctx,
                                lhsT=v_sb[:, jkt, 65 * h:65 * h + 65],
                                rhs=jpt[:, ts(col, 512)],
                                start=(jkt == 0), stop=(jkt == nkt - 1))

                    def emit_norm_pair(jqc, jhp, jctxA, jctxB):
                        if os.environ.get("ABL_NONORM"):
                            return
                        recip = npool.tile([1, 1024], F32, name="recip")
                        nc.vector.reciprocal(recip[:, 0:512],
                                             jctxA[64:65, :])
                        nc.vector.reciprocal(recip[:, 512:1024],
                                             jctxB[64:65, :])
                        rbc = npool.tile([64, 1024], F32, name="rbc")
                        nc.gpsimd.partition_broadcast(rbc, recip)
                        for h, jctx, col in ((2 * jhp, jctxA, 0),
                                             (2 * jhp + 1, jctxB, 1)):
                            nc.vector.tensor_tensor(
                                out=ctxT[jhp][(h % 2) * 64:(h % 2) * 64 + 64,
                                              ds(jqc * 512, 512)],
                                in0=jctx[0:64, :], in1=rbc[:, ts(col, 512)],
                                op=MULT)

                    def emit_outproj_unit(ti):
                        o_sb = outp.tile([128, HID], BF16, name="o_sb")
                        for oc in range(2):
                            o_ps = psS.tile([128, 512], F32, name="o_ps",
                                            tag="s_ps")
                            for hc in range(2):
                                nc.tensor.matmul(
                                    o_ps, lhsT=ctxT[hc][:, ts(ti, 128)],
                                    rhs=wo_sb[:, hc, ts(oc, 512)],
                                    start=(hc == 0), stop=(hc == 1))
                            if os.environ.get("OUT_DVE"):
                                nc.vector.tensor_copy(o_sb[:, ts(oc, 512)],
                                                      o_ps)
                            elif os.environ.get("OUT_SPLIT") and oc == 1:
                                nc.vector.tensor_copy(o_sb[:, ts(oc, 512)],
                                                      o_ps)
                            else:
                                nc.scalar.copy(o_sb[:, ts(oc, 512)], o_ps)
                        if not os.environ.get("NO_OUT_DMA"):
                            nc.sync.dma_start(out[ts(ti, 128)], o_sb)

                    from collections import deque
                    kchunks = []
                    off = 0
                    while off < KC:
                        w = min(512, KC - off)
                        kchunks.append((off, w))
                        off += w

                    # prologue: K chunk 0 + Q chunk 0 (needed at slot 0)
                    for s in range(2):
                        emit_qk(k_il, wk_sb, xk_sb, s, 0, 512, 2 + s)
                    for s in range(2):
                        emit_qk(q_il, wq_sb, xt_sb, s, 0, 512, s)

                    # remaining QKV work, drained into the kt loop by need
                    qkv_jobs = []   # (need_slot, fn)
                    for i in range(nkt):
                        qkv_jobs.append(
                            (i - 1, (lambda i_=i: emit_v(i_))))
                    for ci in range(1, len(kchunks)):
                        off, w = kchunks[ci]
                        for s in range(2):
                            qkv_jobs.append(
                                (4 * ci - 3,
                                 (lambda s_=s, o_=off, w_=w:
                                  emit_qk(k_il, wk_sb, xk_sb, s_, o_, w_,
                                          2 + s_))))
                    for cc in range(1, 4):
                        for s in range(2):
                            qkv_jobs.append(
                                (18 * cc - 5,
                                 (lambda s_=s, o_=512 * cc:
                                  emit_qk(q_il, wq_sb, xt_sb, s_, o_, 512,
                                          s_))))
                    qkv_jobs.sort(key=lambda j: j[0])
                    qkv_jobs = deque(qkv_jobs)
                    look = int(os.environ.get("QKV_LOOK", "4"))
                    jps = int(os.environ.get("QKV_JPS", "2"))

                    deferred = deque()
                    pv_q = deque()
                    pv_lag = int(os.environ.get("PV_LAG", "2"))
                    out_mod = int(os.environ.get("OUT_MOD", "99"))
                    slot = 0
                    due = []        # (due_slot, fn)

                    def run_due():
                        for item in [d for d in due if d[0] <= slot]:
                            due.remove(item)
                            item[1]()

                    escale = 0.125 / (QS * QS)
                    for qc in range(4):          # 512-token query chunks
                        for hp in range(2):      # head pairs (2hp, 2hp+1)
                            ctxA = psX.tile([65, 512], F32, name="ctx_ps")
                            ctxB = psX.tile([65, 512], F32, name="ctx_ps")
                            for kt in range(nkt):
                                nj = 0
                                while (qkv_jobs and nj < jps
                                       and qkv_jobs[0][0] <= slot + look):
                                    qkv_jobs.popleft()[1]()
                                    nj += 1
                                s_ps = psS.tile([128, 1024], F32,
                                                name="s_ps")
                                for col, h in ((0, 2 * hp),
                                               (1, 2 * hp + 1)):
                                    if SCORES_BF16:
                                        hr = 64 * (h % 2)
                                        nc.tensor.matmul(
                                            s_ps[:, ts(col, 512)],
                                            lhsT=k_il[hr:hr + 64, hp,
                                                      ts(kt, 128)],
                                            rhs=q_il[hr:hr + 64, hp,
                                                     ds(qc * 512, 512)],
                                            start=True, stop=True,
                                            tile_position=(hr, 0))
                                    else:
                                        nc.tensor.matmul(
                                            s_ps[:, ts(col, 512)],
                                            lhsT=k_il[32 * h:32 * h + 32, :,
                                                      ts(kt, 128)],
                                            rhs=q_il[32 * h:32 * h + 32, :,
                                                     ds(qc * 512, 512)],
                                            start=True, stop=True,
                                            perf_mode=DR,
                                            tile_position=(32 * h, 0))
                                pt = ptp.tile([128, 1024], BF16, name="pt")
                                nc.scalar.activation(
                                    pt, s_ps, EXP,
                                    bias=maskb_sb[:, kt:kt + 1],
                                    scale=escale)
                                pv_q.append((hp, ctxA, ctxB, pt, kt, qc))
                                if len(pv_q) > pv_lag:
                                    job = pv_q.popleft()
                                    emit_pv(job)
                                    if job[4] == nkt - 1:
                                        jhp, jA, jB, jqc = (job[0], job[1],
                                                            job[2], job[5])
                                        due.append(
                                            (slot + 2,
                                             (lambda a=jqc, b=jhp,
                                              c=jA, d=jB:
                                              emit_norm_pair(a, b, c, d))))
                                        if jhp == 1:
                                            due.append(
                                                (slot + 3,
                                                 (lambda p_=jqc:
                                                  deferred.extend(
                                                      4 * p_ + i
                                                      for i in range(4)))))
                                run_due()
                                if deferred and kt % out_mod == out_mod - 1:
                                    emit_outproj_unit(deferred.popleft())
                                slot += 1
                    # tail
                    while qkv_jobs:
                        qkv_jobs.popleft()[1]()
                    while pv_q:
                        job = pv_q.popleft()
                        emit_pv(job)
                        if job[4] == nkt - 1:
                            jhp, jA, jB, jqc = (job[0], job[1], job[2],
                                                job[5])
                            due.append((slot, (lambda a=jqc, b=jhp, c=jA,
                                               d=jB:
                                               emit_norm_pair(a, b, c, d))))
                    slot += 1000
                    run_due()
                    for i in range(4):
                        deferred.append(12 + i)
                    while deferred:
                        emit_outproj_unit(deferred.popleft())

    nc.compile()
    return nc


_NC = None


def shard_inputs(x, mask, qkv_w, qkv_b, out_w):
    bf = ml_dtypes.bfloat16
    # DoubleRow subtile column permutation: psum partition m, subtile s
    # -> head m//32, dim 32*s + m%32 (local to this core's 4 heads)
    m = np.arange(128)
    if SCORES_BF16:
        perm = np.stack([128 * s_ + m for s_ in range(2)])
    else:
        perm = np.stack([64 * (m // 32) + 32 * s_ + (m % 32)
                         for s_ in range(2)])
    in_maps = []
    for c in range(N_CORES):
        b, g = c // 4, c % 4
        xtb_ = np.ascontiguousarray(x[b].T).astype(bf)
        idx = np.where(mask[b] != 0)[0]
        assert len(idx) <= KC, f"unmasked {len(idx)} > KC={KC}"
        pad = np.zeros(KC - len(idx), np.int64)
        idxp = np.concatenate([idx, pad])
        xkb_ = np.ascontiguousarray(x[b][idxp].T).astype(bf)
        mbk = np.full(KC, np.float32(NEG), np.float32)
        mbk[:len(idx)] = 0.0
        mbk = np.ascontiguousarray(mbk.reshape(NKT, 128).T).astype(np.float32)
        qp = 256 * g + perm              # [2, 128] global q dims
        kp = 1024 + 256 * g + perm
        wq = np.ascontiguousarray(
            np.transpose(qkv_w[qp, :], (2, 0, 1))).astype(bf)
        wk = np.ascontiguousarray(
            np.transpose(qkv_w[kp, :], (2, 0, 1))).astype(bf)
        wv = np.ascontiguousarray(
            qkv_w[2048 + 256 * g:2048 + 256 * (g + 1), :].T).astype(bf)
        qkbc = np.stack([qkv_b[qp[0]], qkv_b[qp[1]],
                         qkv_b[kp[0]], qkv_b[kp[1]]],
                        axis=1).astype(np.float32) * np.float32(QS)
        vbc = qkv_b[2048 + 256 * g:2048 + 256 * (g + 1)][None, :].astype(bf)
        wo = np.ascontiguousarray(
            out_w[:, 256 * g:256 * (g + 1)].T).astype(bf)
        in_maps.append({"xtb": xtb_, "xkb": xkb_,
                        "wqT": wq, "wkT": wk, "wvT": wv,
                        "qkb": qkbc, "vb": vbc,
                        "woT": wo, "maskb": mbk})
    return in_maps


def run(in_maps, **kwargs):
    global _NC
    if _NC is None:
        _NC = build_program()
    return bass_utils.run_bass_kernel_spmd(
        _NC, in_maps, core_ids=list(range(N_CORES)), **kwargs)


def kernel(x, mask, qkv_w, qkv_b, out_w, out_b):
    global KC, NKT, _NC
    x = np.asarray(x)
    mask = np.asarray(mask)
    need = int(np.max(np.sum(mask != 0, axis=1)))
    kc = max(128, ((need + 127) // 128) * 128)
    if kc != KC:
        KC, NKT = kc, kc // 128
        _NC = None
    qkv_w = np.asarray(qkv_w)
    qkv_b = np.asarray(qkv_b)
    out_w = np.asarray(out_w)
    out_b = np.asarray(out_b)
    in_maps = shard_inputs(x, mask, qkv_w, qkv_b, out_w)
    res = run(in_maps)
    parts = [r["out"] for r in res.results]
    full = np.empty((2, S, HID), np.float32)
    for b in range(2):
        acc = parts[4 * b].astype(np.float32)
        for g in range(1, 4):
            acc = acc + parts[4 * b + g].astype(np.float32)
        full[b] = acc + out_b[None, :]
    return full
